# revision 2
# baseline (speedup 1.0000x reference)
"""Bass/Trainium2 kernel for nn_BitwiseTasNetRepeat.

Strategy (v5: rebalanced engine LP + GPSIMD R-offload)
------------------------------------------------------
Every sign(BN(.)) collapses to a per-channel threshold compare. Per block:

    u  = (R >= t1)                          (DVE is_ge, fp8 {0,1}; full-T
                                             wide, 2x_2P mode)
    P  = (2*w1s) @ u                        (TensorE fp8 DoubleRow, K=256;
                                             mh-outer order for weight reuse)
    G  = Sign(-P + t2 + rowsum(w1s))        (ACT, fp8 {-1,0,1});  G = -s2

  depthwise, mh tile 0 ("3-tap" path, S3 on ACT):
    q  = -a0*G(-d) - G(0) - a2*G(+d)        (TensorE: DR pair (t-d,t) +
                                             plain tap (t+d), 2 MMs)
    S3 = Sign(q - tau)                      (ACT, fp8 {-1,1})

  depthwise, mh tiles 1..3 ("2-tap" path, S3 on DVE):
    q2 = -a0*G(-d) - a2*G(+d)               (TensorE: ONE DR diag matmul,
                                             rhs pair-AP stride 2d)
    S3 = ((q2 - tau) >= G(0))               (DVE scalar_tensor_tensor,
                                             fp8 {0,1}) -- center tap
                                             folded into the compare

    P2 = w2' @ S3                           (TensorE fp8 DR, K=512; w2' is
                                             W2x for 3-tap rows, 2*W2x for
                                             2-tap rows)
    R  = (P2 - rowsum2) + R                 (most chunks: DVE STT; some
                                             chunks: ACT Identity copy with
                                             bias -rowsum2 to SBUF tmp, then
                                             GPSIMD tensor_tensor add)

G tiles live in a 10-slot ring of bufs=1 pool tags so the dilation halos
are zeroed exactly once (zeros persist across ring reuse).

Sharding: data-parallel over batch, 2 batches per core, 8 cores.
"""

import numpy as np
import ml_dtypes

_B, _CB, _H, _T = 16, 256, 512, 4096
_BLOCKS = 8
_EPS = 1e-5
_NCORES = 8
_BS = _B // _NCORES      # batches per core
_KC = _CB // 128         # 2  k-tiles of Cb
_MH = _H // 128          # 4  m-tiles of H
_NMH3 = 1                # mh tiles using the 3-tap/ACT-S3 path
_PAD = 128               # halo for dilated depthwise conv (max d = 128)
_NCC = 14                # f32 const columns per block
_QC = 1024               # chunk width for dw/S3/conv2 stages
_NGR = 10                # G ring slots
# (mc, g) R-update chunks routed via ACT-copy + GPSIMD add instead of DVE
_RALT = {(0, 1), (1, 2), (0, 3)}

_nc_cache = {}


def _mk3(ap2d, j_step, cols):
    """3D AP [128, 2 (stride j_step), cols] over a 2D row view."""
    import bass_rust
    v = ap2d.copy()
    l = v.ap
    v.ap = bass_rust.VecI64Pair([list(l[0]), [j_step, 2], [1, cols]])
    return v


def _build_nc(bs=_BS, nblocks=_BLOCKS, T=_T):
    import concourse.mybir as mybir
    from concourse import bacc
    from concourse.tile import TileContext

    f32 = mybir.dt.float32
    fp8 = mybir.dt.float8e4
    ALU = mybir.AluOpType
    DRM = mybir.MatmulPerfMode.DoubleRow
    SIGN = mybir.ActivationFunctionType.Sign
    IDENT = mybir.ActivationFunctionType.Identity
    nq = T // _QC

    nc = bacc.Bacc("TRN2", target_bir_lowering=False, debug=False,
                   enable_asserts=False)

    x_d = nc.dram_tensor("x", [bs, _CB, T], f32, kind="ExternalInput")
    w1_d = nc.dram_tensor("w1dr", [128, nblocks * _MH * 256], fp8,
                          kind="ExternalInput")
    w2_d = nc.dram_tensor("w2dr", [128, nblocks * _KC * 2 * 256], fp8,
                          kind="ExternalInput")
    dwp_d = nc.dram_tensor("dwp", [128, nblocks * _MH * 256], fp8,
                           kind="ExternalInput")
    dw2_d = nc.dram_tensor("dw2", [128, nblocks * _NMH3 * 128], fp8,
                           kind="ExternalInput")
    cst_d = nc.dram_tensor("cst", [128, nblocks * _NCC], f32,
                           kind="ExternalInput")
    out_d = nc.dram_tensor("out", [bs, _CB, T], f32, kind="ExternalOutput")

    with TileContext(nc) as tc:
        with (
            tc.tile_pool(name="wpool", bufs=1) as wpool,
            tc.tile_pool(name="rpool", bufs=4) as rpool,
            tc.tile_pool(name="s1pool", bufs=4) as s1pool,
            tc.tile_pool(name="s2pool", bufs=1) as s2pool,
            tc.tile_pool(name="s3pool", bufs=8) as s3pool,
            tc.tile_pool(name="tpool", bufs=3) as tpool,
            tc.tile_pool(name="psmm", bufs=2, space="PSUM") as psmm,
            tc.tile_pool(name="psdw", bufs=2, space="PSUM") as psdw,
        ):
            w1sb = wpool.tile([128, nblocks * _MH * 256], fp8)
            nc.sync.dma_start(out=w1sb[:], in_=w1_d.ap())
            w2sb = wpool.tile([128, nblocks * _KC * 2 * 256], fp8)
            nc.sync.dma_start(out=w2sb[:], in_=w2_d.ap())
            dwpsb = wpool.tile([128, nblocks * _MH * 256], fp8)
            nc.sync.dma_start(out=dwpsb[:], in_=dwp_d.ap())
            dw2sb = wpool.tile([128, nblocks * _NMH3 * 128], fp8)
            nc.sync.dma_start(out=dw2sb[:], in_=dw2_d.ap())
            cst = wpool.tile([128, nblocks * _NCC], f32)
            nc.sync.dma_start(out=cst[:], in_=cst_d.ap())

            def w1t(i, mh):
                o = (i * _MH + mh) * 256
                return _mk3(w1sb[:, o:o + 256], 128, 128)

            def w2t(i, mc, pair):
                o = (i * _KC * 2 + mc * 2 + pair) * 256
                return _mk3(w2sb[:, o:o + 256], 128, 128)

            def dwt(i, mh):
                o = (i * _MH + mh) * 256
                return _mk3(dwpsb[:, o:o + 256], 128, 128)

            def dw2t(i, mh):
                o = (i * _NMH3 + mh) * 128
                return dw2sb[:, o:o + 128]

            def cc(i, j):
                return cst[:, i * _NCC + j:i * _NCC + j + 1]

            # PE warmup: a few junk matmuls so HAM reaches K=8/8 before the
            # real stream starts (weights tile reused as dummy data). Lives
            # in psdw whose first real tile isn't needed until stage BC.
            wu = psdw.tile([128, _QC], f32, tag="dw", name="warmup")
            for _ in range(12):
                nc.tensor.matmul(wu[:, 0:512], w1sb[:, 0:128],
                                 w1sb[:, 0:512], start=True, stop=True)

            Rb = {}
            for b in range(bs):
                Rb[b] = []
                for kc in range(_KC):
                    rt = rpool.tile([128, T], f32, tag="R",
                                    name=f"R_b{b}_{kc}")
                    Rb[b].append(rt)
            # chunk-granular loads, first chunks of every tile up front so
            # the first S1 (needs both kc tiles) starts early
            for g in range(nq):
                for b in range(bs):
                    for kc in range(_KC):
                        nc.sync.dma_start(
                            out=Rb[b][kc][:, g * _QC:(g + 1) * _QC],
                            in_=x_d.ap()[b, kc * 128:(kc + 1) * 128,
                                         g * _QC:(g + 1) * _QC])

            state = {}
            ginit = [False] * _NGR
            gctr = [0]

            def emitA_alloc(b, i):
                # full-width S1 for both kc tiles (R is final: written two
                # steps back in the b-inner sequence)
                S1 = s1pool.tile([128, _KC * T], fp8, tag="S1",
                                 name=f"S1_b{b}_i{i}")
                R = Rb[b]
                for kc in range(_KC):
                    nc.vector.tensor_scalar(
                        S1[:, kc * T:(kc + 1) * T], R[kc][:],
                        cc(i, kc), None, op0=ALU.is_ge)
                G = []
                for mh in range(_MH):
                    slot = gctr[0] % _NGR
                    gctr[0] += 1
                    gt = s2pool.tile([128, T + 2 * _PAD], fp8,
                                     tag=f"S2_{slot}",
                                     name=f"G_b{b}_i{i}_{mh}")
                    if not ginit[slot]:
                        ginit[slot] = True
                        nc.gpsimd.memset(gt[:, 0:_PAD], 0.0)
                        nc.gpsimd.memset(gt[:, _PAD + T:2 * _PAD + T], 0.0)
                    G.append(gt)
                state[(b, i)] = (S1, G)

            def emitA_mh(b, i, mh):
                # conv1 + G threshold for one mh tile over all chunks:
                # consecutive matmuls share the same stationary weights
                S1, G = state[(b, i)]
                for g in range(nq):
                    c0 = g * _QC
                    ps = psmm.tile([128, _QC], f32, tag="mm",
                                   name=f"psA_{b}_{i}_{mh}_{g}")
                    for nn in range(2):
                        cn = c0 + nn * 512
                        rhs = _mk3(S1[:, cn:cn + 512], T, 512)
                        nc.tensor.matmul(
                            ps[:, nn * 512:(nn + 1) * 512],
                            w1t(i, mh), rhs, start=True, stop=True,
                            perf_mode=DRM)
                    # G = Sign(-P + t2 + rowsum(w1s)) = -s2
                    nc.scalar.activation(
                        G[mh][:, _PAD + c0:_PAD + c0 + _QC],
                        ps[:], SIGN, bias=cc(i, 2 + mh), scale=-1.0)

            def emitBC_q(b, i, q):
                d = 2 ** i
                R = Rb[b]
                _, G = state[(b, i)]
                c0 = q * _QC
                S3 = [s3pool.tile([128, 2 * _QC], fp8, tag="S3",
                                  name=f"S3_b{b}_i{i}_q{q}_p{p}")
                      for p in range(2)]
                for mh in range(_MH):
                    pd = psdw.tile([128, _QC], f32, tag="dw",
                                   name=f"psD_{b}_{i}_q{q}_{mh}")
                    s3out = S3[mh // 2][:, (mh % 2) * _QC:(mh % 2 + 1) * _QC]
                    if mh < _NMH3:
                        # 3-tap: q = -a0*G(-d) - G(0) - a2*G(+d) in PSUM
                        for nn in range(2):
                            w0 = _PAD + c0 + nn * 512
                            rhs01 = _mk3(G[mh][:, w0 - d:w0 - d + 512], d, 512)
                            nc.tensor.matmul(
                                pd[:, nn * 512:(nn + 1) * 512],
                                dwt(i, mh), rhs01,
                                start=True, stop=False, perf_mode=DRM)
                            nc.tensor.matmul(
                                pd[:, nn * 512:(nn + 1) * 512],
                                dw2t(i, mh),
                                G[mh][:, w0 + d:w0 + d + 512],
                                start=False, stop=True)
                        # S3 = Sign(q - tau) in {-1,1}
                        nc.scalar.activation(
                            s3out, pd[:], SIGN, bias=cc(i, 6 + mh))
                    else:
                        # 2-tap: q2 = -a0*G(-d) - a2*G(+d), one DR matmul
                        for nn in range(2):
                            w0 = _PAD + c0 + nn * 512
                            rhs = _mk3(G[mh][:, w0 - d:w0 - d + 512],
                                       2 * d, 512)
                            nc.tensor.matmul(
                                pd[:, nn * 512:(nn + 1) * 512],
                                dwt(i, mh), rhs,
                                start=True, stop=True, perf_mode=DRM)
                        # S3 = ((q2 - tau) >= G(0)) in {0,1}
                        nc.vector.scalar_tensor_tensor(
                            s3out, pd[:], cc(i, 6 + mh),
                            G[mh][:, _PAD + c0:_PAD + c0 + _QC],
                            op0=ALU.subtract, op1=ALU.is_ge)
                for mc in range(_KC):
                    ps2 = psmm.tile([128, _QC], f32, tag="mm",
                                    name=f"psC_{b}_{i}_q{q}_{mc}")
                    for nn in range(2):
                        for pair in range(2):
                            rhs = _mk3(S3[pair][:, nn * 512:nn * 512 + 512],
                                       _QC, 512)
                            nc.tensor.matmul(
                                ps2[:, nn * 512:(nn + 1) * 512],
                                w2t(i, mc, pair), rhs,
                                start=(pair == 0), stop=(pair == 1),
                                perf_mode=DRM)
                    if (mc, q) in _RALT:
                        # R update off DVE: tmp = P2 - rowsum2 (ACT), then
                        # R += tmp (GPSIMD)
                        tmp = tpool.tile([128, _QC], f32, tag="tmp",
                                         name=f"tp_{b}_{i}_q{q}_{mc}")
                        nc.scalar.activation(tmp[:], ps2[:], IDENT,
                                             bias=cc(i, 12 + mc), scale=1.0)
                        nc.gpsimd.tensor_tensor(
                            R[mc][:, c0:c0 + _QC], R[mc][:, c0:c0 + _QC],
                            tmp[:], op=ALU.add)
                    else:
                        # R = (P2 - rowsum2) + R
                        nc.vector.scalar_tensor_tensor(
                            R[mc][:, c0:c0 + _QC], ps2[:], cc(i, 10 + mc),
                            R[mc][:, c0:c0 + _QC],
                            op0=ALU.subtract, op1=ALU.add)
                    if i == _BLOCKS - 1:
                        # final block: stream the finished chunk out now so
                        # the output DMA drains during the remaining compute
                        nc.sync.dma_start(
                            out=out_d.ap()[b, mc * 128:(mc + 1) * 128,
                                           c0:c0 + _QC],
                            in_=R[mc][:, c0:c0 + _QC])

            # software-pipelined emission: stage A of step k+1 interleaves
            # unit-wise with stage B/C of step k so every engine streams
            seq = [(b, i) for i in range(nblocks) for b in range(bs)]
            emitA_alloc(*seq[0])
            for mh in range(_MH):
                emitA_mh(*seq[0], mh)
            for k in range(len(seq)):
                if k + 1 < len(seq):
                    emitA_alloc(*seq[k + 1])
                for j in range(nq):
                    # stage A of step k+1 first: its S1/conv1/G chain is
                    # ready (deps are k-1) and unblocks the other engines
                    if k + 1 < len(seq):
                        emitA_mh(*seq[k + 1], j)
                    emitBC_q(*seq[k], j)
                state.pop(seq[k])

    nc.finalize()
    return nc


def _prep(inputs, nblocks=_BLOCKS):
    """Host-side weight/threshold preprocessing (tiny tensors only)."""
    e4 = ml_dtypes.float8_e4m3

    def thr(g, bb, m, v):
        return (m - bb * np.sqrt(v + _EPS) / g).astype(np.float32)

    w1dr = np.zeros((128, nblocks * _MH * 256), np.float32)
    w2dr = np.zeros((128, nblocks * _KC * 2 * 256), np.float32)
    dwp = np.zeros((128, nblocks * _MH * 256), np.float32)
    dw2 = np.zeros((128, nblocks * _NMH3 * 128), np.float32)
    cst = np.zeros((128, nblocks * _NCC), np.float32)
    for i in range(nblocks):
        t1 = thr(inputs['bn1_gamma'][i], inputs['bn1_beta'][i],
                 inputs['bn1_mean'][i], inputs['bn1_var'][i])          # [Cb]
        t2 = thr(inputs['bn2_gamma'][i], inputs['bn2_beta'][i],
                 inputs['bn2_mean'][i], inputs['bn2_var'][i])          # [H]
        t3 = thr(inputs['bn3_gamma'][i], inputs['bn3_beta'][i],
                 inputs['bn3_mean'][i], inputs['bn3_var'][i])          # [H]
        W1s = np.sign(inputs['w1'][i]).astype(np.float32)              # [H, Cb]
        W2s = np.sign(inputs['w2'][i]).astype(np.float32)              # [Cb, H]
        dws = np.sign(inputs['dw_w'][i]).astype(np.float32)            # [H, 3]
        ctr = dws[:, 1]
        a0 = dws[:, 0] * ctr
        a2 = dws[:, 2] * ctr
        W2x = W2s * ctr[None, :]                                       # [Cb, H]
        rw1 = W1s.sum(axis=1)                                          # [H]
        # conv2 weight scale: 3-tap rows (S3 in {-1,1}) use W2x; 2-tap rows
        # (S3 in {0,1}) use 2*W2x with a rowsum correction
        hsc = np.where(np.arange(_H) < _NMH3 * 128, 1.0, 2.0)
        W2p = W2x * hsc[None, :]
        rw2 = W2x[:, _NMH3 * 128:].sum(axis=1)                         # [Cb]
        for mh in range(_MH):
            o = (i * _MH + mh) * 256
            for j in range(2):
                # w1dr[p, j*128+f] = 2*W1s[mh*128+f, j*128+p]
                w1dr[:, o + j * 128:o + (j + 1) * 128] = \
                    2.0 * W1s[mh * 128:(mh + 1) * 128,
                              j * 128:(j + 1) * 128].T
        for mc in range(_KC):
            for pair in range(2):
                o = (i * _KC * 2 + mc * 2 + pair) * 256
                for j in range(2):
                    kh = pair * 2 + j
                    w2dr[:, o + j * 128:o + (j + 1) * 128] = \
                        W2p[mc * 128:(mc + 1) * 128,
                            kh * 128:(kh + 1) * 128].T
        for mh in range(_MH):
            sl = slice(mh * 128, (mh + 1) * 128)
            o = (i * _MH + mh) * 256
            if mh < _NMH3:
                # 3-tap: DR pair (t-d, t) weights (-a0, -1); plain (t+d): -a2
                dwp[np.arange(128), o + np.arange(128)] = -a0[sl]
                dwp[np.arange(128), o + 128 + np.arange(128)] = -1.0
                o2 = (i * _NMH3 + mh) * 128
                dw2[np.arange(128), o2 + np.arange(128)] = -a2[sl]
            else:
                # 2-tap: DR pair (t-d, t+d) weights (-a0, -a2)
                dwp[np.arange(128), o + np.arange(128)] = -a0[sl]
                dwp[np.arange(128), o + 128 + np.arange(128)] = -a2[sl]
        base = i * _NCC
        for kc in range(_KC):
            cst[:, base + kc] = t1[kc * 128:(kc + 1) * 128]
        tau3 = ctr * t3
        for mh in range(_MH):
            sl = slice(mh * 128, (mh + 1) * 128)
            cst[:, base + 2 + mh] = t2[sl] + rw1[sl]       # ACT Sign bias
            # 3-tap mh: ACT Sign bias = -tau ; 2-tap mh: STT scalar = tau
            cst[:, base + 6 + mh] = (-tau3[sl] if mh < _NMH3 else tau3[sl])
        for mc in range(_KC):
            cst[:, base + 10 + mc] = rw2[mc * 128:(mc + 1) * 128]
            cst[:, base + 12 + mc] = -rw2[mc * 128:(mc + 1) * 128]
    return (w1dr.astype(e4), w2dr.astype(e4), dwp.astype(e4),
            dw2.astype(e4), cst)


def kernel(**inputs):
    inputs = {k: np.asarray(v) for k, v in inputs.items()}
    x = inputs['x'].astype(np.float32)
    w1dr, w2dr, dwp, dw2, cst = _prep(inputs)

    if 'nc' not in _nc_cache:
        _nc_cache['nc'] = _build_nc()
    nc = _nc_cache['nc']

    in_maps = []
    for c in range(_NCORES):
        in_maps.append({
            'x': np.ascontiguousarray(x[c * _BS:(c + 1) * _BS]),
            'w1dr': w1dr, 'w2dr': w2dr, 'dwp': dwp, 'dw2': dw2,
            'cst': cst,
        })

    from concourse.bass_utils import run_bass_kernel_spmd
    import os
    trace = bool(int(os.environ.get('KERNEL_TRACE', '0')))
    res = run_bass_kernel_spmd(nc, in_maps, core_ids=list(range(_NCORES)),
                               trace=trace)
    _nc_cache['last_result'] = res
    out = np.concatenate([r['out'] for r in res.results], axis=0)
    return out.astype(np.float32)


# revision 10
# speedup vs baseline: 1.0158x; 1.0158x over previous
"""Bass/Trainium2 kernel for nn_BitwiseTasNetRepeat.

Strategy (v4c: balanced 3-engine split of the threshold passes)
---------------------------------------------------------------
Every sign(BN(.)) collapses to a per-channel threshold compare. Per block:

    u  = (R >= t1)                          (DVE is_ge, fp8 {0,1};
                                             single-src op -> 2x_2P mode)
    P  = (2*w1s) @ u                        (TensorE fp8 DoubleRow, K=256)
    G  = Sign(-P + t2 + rowsum(w1s))        (ACT, fp8 {-1,0,1});  G = -s2

  depthwise, mh tiles 0..NMH3-1 ("3-tap" path, S3 on ACT):
    q  = -a0*G(-d) - G(0) - a2*G(+d)        (TensorE: DR pair (t-d,t) +
                                             plain tap (t+d), 2 MMs)
    S3 = Sign(q - tau)                      (ACT, fp8 {-1,1})

  depthwise, mh tiles NMH3..3 ("2-tap" path, S3 on DVE):
    q2 = -a0*G(-d) - a2*G(+d)               (TensorE: ONE DR diag matmul,
                                             rhs pair-AP stride 2d)
    S3 = ((q2 - tau) >= G(0))               (DVE scalar_tensor_tensor,
                                             fp8 {0,1}) -- center tap
                                             folded into the compare

    P2 = w2' @ S3                           (TensorE fp8 DR, K=512; w2' is
                                             W2x for 3-tap rows, 2*W2x for
                                             2-tap rows)
    R  = (P2 - rowsum2) + R                 (DVE scalar_tensor_tensor)

with a0 = sign(dw0)*ctr, a2 = sign(dw2)*ctr, ctr = sign(dw1) folded into
W2x = sign(w2)*ctr and tau = ctr*t3.  All values are exact in fp8/f32-PSUM.

Sharding: data-parallel over batch, 2 batches per core, 8 cores.
"""

import numpy as np
import ml_dtypes

_B, _CB, _H, _T = 16, 256, 512, 4096
_BLOCKS = 8
_EPS = 1e-5
_NCORES = 8
_BS = _B // _NCORES      # batches per core
_KC = _CB // 128         # 2  k-tiles of Cb
_MH = _H // 128          # 4  m-tiles of H
_NMH3 = 2                # max mh tiles on the 3-tap/ACT-S3 path (dw2 sizing)
# per-block 3-tap tile count: avg 1.75 rebalances TensorE (dw passes) vs
# DVE (2-tap STT) vs ACT (3-tap Sign) toward the LP optimum
_NMH3S = [2, 2, 2, 1, 2, 1, 2, 2]
_PAD = 128               # halo for dilated depthwise conv (max d = 128)
_NCC = 12                # f32 const columns per block
_QC = 1024               # chunk width for dw/S3/conv2 stages

_nc_cache = {}


def _mk3(ap2d, j_step, cols):
    """3D AP [128, 2 (stride j_step), cols] over a 2D row view."""
    import bass_rust
    v = ap2d.copy()
    l = v.ap
    v.ap = bass_rust.VecI64Pair([list(l[0]), [j_step, 2], [1, cols]])
    return v


def _build_nc(bs=_BS, nblocks=_BLOCKS, T=_T):
    import concourse.mybir as mybir
    from concourse import bacc
    from concourse.tile import TileContext

    f32 = mybir.dt.float32
    fp8 = mybir.dt.float8e4
    ALU = mybir.AluOpType
    DRM = mybir.MatmulPerfMode.DoubleRow
    SIGN = mybir.ActivationFunctionType.Sign
    nq = T // _QC

    nc = bacc.Bacc("TRN2", target_bir_lowering=False, debug=False,
                   enable_asserts=False)

    x_d = nc.dram_tensor("x", [bs, _CB, T], f32, kind="ExternalInput")
    w1_d = nc.dram_tensor("w1dr", [128, nblocks * _MH * 256], fp8,
                          kind="ExternalInput")
    w2_d = nc.dram_tensor("w2dr", [128, nblocks * _KC * 2 * 256], fp8,
                          kind="ExternalInput")
    dwp_d = nc.dram_tensor("dwp", [128, nblocks * _MH * 256], fp8,
                           kind="ExternalInput")
    dw2_d = nc.dram_tensor("dw2", [128, nblocks * _NMH3 * 128], fp8,
                           kind="ExternalInput")
    cst_d = nc.dram_tensor("cst", [128, nblocks * _NCC], f32,
                           kind="ExternalInput")
    out_d = nc.dram_tensor("out", [bs, _CB, T], f32, kind="ExternalOutput")

    with TileContext(nc) as tc:
        with (
            tc.tile_pool(name="wpool", bufs=1) as wpool,
            tc.tile_pool(name="rpool", bufs=4) as rpool,
            tc.tile_pool(name="s1pool", bufs=4) as s1pool,
            tc.tile_pool(name="s2pool", bufs=10) as s2pool,
            tc.tile_pool(name="s3pool", bufs=8) as s3pool,
            tc.tile_pool(name="psmm", bufs=2, space="PSUM") as psmm,
            tc.tile_pool(name="psdw", bufs=2, space="PSUM") as psdw,
        ):
            w1sb = wpool.tile([128, nblocks * _MH * 256], fp8)
            nc.sync.dma_start(out=w1sb[:], in_=w1_d.ap())
            w2sb = wpool.tile([128, nblocks * _KC * 2 * 256], fp8)
            nc.sync.dma_start(out=w2sb[:], in_=w2_d.ap())
            dwpsb = wpool.tile([128, nblocks * _MH * 256], fp8)
            nc.sync.dma_start(out=dwpsb[:], in_=dwp_d.ap())
            dw2sb = wpool.tile([128, nblocks * _NMH3 * 128], fp8)
            nc.sync.dma_start(out=dw2sb[:], in_=dw2_d.ap())
            cst = wpool.tile([128, nblocks * _NCC], f32)
            nc.sync.dma_start(out=cst[:], in_=cst_d.ap())

            def w1t(i, mh):
                o = (i * _MH + mh) * 256
                return _mk3(w1sb[:, o:o + 256], 128, 128)

            def w2t(i, mc, pair):
                o = (i * _KC * 2 + mc * 2 + pair) * 256
                return _mk3(w2sb[:, o:o + 256], 128, 128)

            def dwt(i, mh):
                o = (i * _MH + mh) * 256
                return _mk3(dwpsb[:, o:o + 256], 128, 128)

            def dw2t(i, mh):
                o = (i * _NMH3 + mh) * 128
                return dw2sb[:, o:o + 128]

            def cc(i, j):
                return cst[:, i * _NCC + j:i * _NCC + j + 1]

            # PE warmup: a few junk matmuls so HAM reaches K=8/8 before the
            # real stream starts (weights tile reused as dummy data). Lives
            # in psdw whose first real tile isn't needed until stage BC.
            wu = psdw.tile([128, _QC], f32, tag="dw", name="warmup")
            for _ in range(12):
                nc.tensor.matmul(wu[:, 0:512], w1sb[:, 0:128],
                                 w1sb[:, 0:512], start=True, stop=True)

            Rb = {}
            for b in range(bs):
                Rb[b] = []
                for kc in range(_KC):
                    rt = rpool.tile([128, T], f32, tag="R",
                                    name=f"R_b{b}_{kc}")
                    Rb[b].append(rt)
            # chunk-granular loads, first chunks of every tile up front so
            # the first S1 (needs both kc tiles' chunk 0) starts early
            for g in range(nq):
                for b in range(bs):
                    for kc in range(_KC):
                        nc.sync.dma_start(
                            out=Rb[b][kc][:, g * _QC:(g + 1) * _QC],
                            in_=x_d.ap()[b, kc * 128:(kc + 1) * 128,
                                         g * _QC:(g + 1) * _QC])

            state = {}

            def emitA_alloc(b, i):
                S1 = s1pool.tile([128, _KC * T], fp8, tag="S1",
                                 name=f"S1_b{b}_i{i}")
                # full-width S1 (R final since two steps back): one DVE
                # 2x_2P op per kc tile instead of four chunk-wise ones
                R = Rb[b]
                for kc in range(_KC):
                    nc.vector.tensor_scalar(
                        S1[:, kc * T:(kc + 1) * T], R[kc][:],
                        cc(i, kc), None, op0=ALU.is_ge)
                G = []
                for mh in range(_MH):
                    gt = s2pool.tile([128, T + 2 * _PAD], fp8, tag="S2",
                                     name=f"G_b{b}_i{i}_{mh}")
                    nc.gpsimd.memset(gt[:, 0:_PAD], 0.0)
                    nc.gpsimd.memset(gt[:, _PAD + T:2 * _PAD + T], 0.0)
                    G.append(gt)
                state[(b, i)] = (S1, G)

            def emitA_g(b, i, g):
                S1, G = state[(b, i)]
                c0 = g * _QC
                for mh in range(_MH):
                    ps = psmm.tile([128, _QC], f32, tag="mm",
                                   name=f"psA_{b}_{i}_{mh}_{g}")
                    for nn in range(2):
                        cn = c0 + nn * 512
                        rhs = _mk3(S1[:, cn:cn + 512], T, 512)
                        nc.tensor.matmul(
                            ps[:, nn * 512:(nn + 1) * 512],
                            w1t(i, mh), rhs, start=True, stop=True,
                            perf_mode=DRM)
                    # G = Sign(-P + t2 + rowsum(w1s)) = -s2
                    nc.scalar.activation(
                        G[mh][:, _PAD + c0:_PAD + c0 + _QC],
                        ps[:], SIGN, bias=cc(i, 2 + mh), scale=-1.0)

            def emitBC_q(b, i, q):
                d = 2 ** i
                R = Rb[b]
                _, G = state[(b, i)]
                c0 = q * _QC
                S3 = [s3pool.tile([128, 2 * _QC], fp8, tag="S3",
                                  name=f"S3_b{b}_i{i}_q{q}_p{p}")
                      for p in range(2)]
                for mh in range(_MH):
                    pd = psdw.tile([128, _QC], f32, tag="dw",
                                   name=f"psD_{b}_{i}_q{q}_{mh}")
                    s3out = S3[mh // 2][:, (mh % 2) * _QC:(mh % 2 + 1) * _QC]
                    if mh < _NMH3S[i]:
                        # 3-tap: q = -a0*G(-d) - G(0) - a2*G(+d) in PSUM
                        for nn in range(2):
                            w0 = _PAD + c0 + nn * 512
                            rhs01 = _mk3(G[mh][:, w0 - d:w0 - d + 512], d, 512)
                            nc.tensor.matmul(
                                pd[:, nn * 512:(nn + 1) * 512],
                                dwt(i, mh), rhs01,
                                start=True, stop=False, perf_mode=DRM)
                            nc.tensor.matmul(
                                pd[:, nn * 512:(nn + 1) * 512],
                                dw2t(i, mh),
                                G[mh][:, w0 + d:w0 + d + 512],
                                start=False, stop=True)
                        # S3 = Sign(q - tau) in {-1,1}
                        nc.scalar.activation(
                            s3out, pd[:], SIGN, bias=cc(i, 6 + mh))
                    else:
                        # 2-tap: q2 = -a0*G(-d) - a2*G(+d), one DR matmul
                        for nn in range(2):
                            w0 = _PAD + c0 + nn * 512
                            rhs = _mk3(G[mh][:, w0 - d:w0 - d + 512],
                                       2 * d, 512)
                            nc.tensor.matmul(
                                pd[:, nn * 512:(nn + 1) * 512],
                                dwt(i, mh), rhs,
                                start=True, stop=True, perf_mode=DRM)
                        # S3 = ((q2 - tau) >= G(0)) in {0,1}
                        nc.vector.scalar_tensor_tensor(
                            s3out, pd[:], cc(i, 6 + mh),
                            G[mh][:, _PAD + c0:_PAD + c0 + _QC],
                            op0=ALU.subtract, op1=ALU.is_ge)
                for mc in range(_KC):
                    ps2 = psmm.tile([128, _QC], f32, tag="mm",
                                    name=f"psC_{b}_{i}_q{q}_{mc}")
                    for nn in range(2):
                        for pair in range(2):
                            rhs = _mk3(S3[pair][:, nn * 512:nn * 512 + 512],
                                       _QC, 512)
                            nc.tensor.matmul(
                                ps2[:, nn * 512:(nn + 1) * 512],
                                w2t(i, mc, pair), rhs,
                                start=(pair == 0), stop=(pair == 1),
                                perf_mode=DRM)
                    # R = (P2 - rowsum2) + R
                    nc.vector.scalar_tensor_tensor(
                        R[mc][:, c0:c0 + _QC], ps2[:], cc(i, 10 + mc),
                        R[mc][:, c0:c0 + _QC],
                        op0=ALU.subtract, op1=ALU.add)
                    if i == _BLOCKS - 1:
                        # final block: stream the finished chunk out now so
                        # the output DMA drains during the remaining compute
                        nc.sync.dma_start(
                            out=out_d.ap()[b, mc * 128:(mc + 1) * 128,
                                           c0:c0 + _QC],
                            in_=R[mc][:, c0:c0 + _QC])

            # software-pipelined emission: stage A of step k+1 interleaves
            # chunk-wise with stage B/C of step k so every engine streams
            seq = [(b, i) for i in range(nblocks) for b in range(bs)]
            emitA_alloc(*seq[0])
            for g in range(nq):
                emitA_g(*seq[0], g)
            for k in range(len(seq)):
                if k + 1 < len(seq):
                    emitA_alloc(*seq[k + 1])
                for g in range(nq):
                    # stage A of step k+1 first: its S1/conv1/G chain is
                    # ready (deps are k-1) and unblocks the other engines
                    if k + 1 < len(seq):
                        emitA_g(*seq[k + 1], g)
                    emitBC_q(*seq[k], g)
                state.pop(seq[k])

    nc.finalize()
    return nc


def _prep(inputs, nblocks=_BLOCKS):
    """Host-side weight/threshold preprocessing (tiny tensors only)."""
    e4 = ml_dtypes.float8_e4m3

    def thr(g, bb, m, v):
        return (m - bb * np.sqrt(v + _EPS) / g).astype(np.float32)

    w1dr = np.zeros((128, nblocks * _MH * 256), np.float32)
    w2dr = np.zeros((128, nblocks * _KC * 2 * 256), np.float32)
    dwp = np.zeros((128, nblocks * _MH * 256), np.float32)
    dw2 = np.zeros((128, nblocks * _NMH3 * 128), np.float32)
    cst = np.zeros((128, nblocks * _NCC), np.float32)
    for i in range(nblocks):
        t1 = thr(inputs['bn1_gamma'][i], inputs['bn1_beta'][i],
                 inputs['bn1_mean'][i], inputs['bn1_var'][i])          # [Cb]
        t2 = thr(inputs['bn2_gamma'][i], inputs['bn2_beta'][i],
                 inputs['bn2_mean'][i], inputs['bn2_var'][i])          # [H]
        t3 = thr(inputs['bn3_gamma'][i], inputs['bn3_beta'][i],
                 inputs['bn3_mean'][i], inputs['bn3_var'][i])          # [H]
        W1s = np.sign(inputs['w1'][i]).astype(np.float32)              # [H, Cb]
        W2s = np.sign(inputs['w2'][i]).astype(np.float32)              # [Cb, H]
        dws = np.sign(inputs['dw_w'][i]).astype(np.float32)            # [H, 3]
        ctr = dws[:, 1]
        a0 = dws[:, 0] * ctr
        a2 = dws[:, 2] * ctr
        W2x = W2s * ctr[None, :]                                       # [Cb, H]
        rw1 = W1s.sum(axis=1)                                          # [H]
        # conv2 weight scale: 3-tap rows (S3 in {-1,1}) use W2x; 2-tap rows
        # (S3 in {0,1}) use 2*W2x with a rowsum correction
        nmh3 = _NMH3S[i]
        hsc = np.where(np.arange(_H) < nmh3 * 128, 1.0, 2.0)
        W2p = W2x * hsc[None, :]
        rw2 = W2x[:, nmh3 * 128:].sum(axis=1)                          # [Cb]
        for mh in range(_MH):
            o = (i * _MH + mh) * 256
            for j in range(2):
                # w1dr[p, j*128+f] = 2*W1s[mh*128+f, j*128+p]
                w1dr[:, o + j * 128:o + (j + 1) * 128] = \
                    2.0 * W1s[mh * 128:(mh + 1) * 128,
                              j * 128:(j + 1) * 128].T
        for mc in range(_KC):
            for pair in range(2):
                o = (i * _KC * 2 + mc * 2 + pair) * 256
                for j in range(2):
                    kh = pair * 2 + j
                    w2dr[:, o + j * 128:o + (j + 1) * 128] = \
                        W2p[mc * 128:(mc + 1) * 128,
                            kh * 128:(kh + 1) * 128].T
        for mh in range(_MH):
            sl = slice(mh * 128, (mh + 1) * 128)
            o = (i * _MH + mh) * 256
            if mh < nmh3:
                # 3-tap: DR pair (t-d, t) weights (-a0, -1); plain (t+d): -a2
                dwp[np.arange(128), o + np.arange(128)] = -a0[sl]
                dwp[np.arange(128), o + 128 + np.arange(128)] = -1.0
                o2 = (i * _NMH3 + mh) * 128
                dw2[np.arange(128), o2 + np.arange(128)] = -a2[sl]
            else:
                # 2-tap: DR pair (t-d, t+d) weights (-a0, -a2)
                dwp[np.arange(128), o + np.arange(128)] = -a0[sl]
                dwp[np.arange(128), o + 128 + np.arange(128)] = -a2[sl]
        base = i * _NCC
        for kc in range(_KC):
            cst[:, base + kc] = t1[kc * 128:(kc + 1) * 128]
        tau3 = ctr * t3
        for mh in range(_MH):
            sl = slice(mh * 128, (mh + 1) * 128)
            cst[:, base + 2 + mh] = t2[sl] + rw1[sl]       # ACT Sign bias
            # 3-tap mh: ACT Sign bias = -tau ; 2-tap mh: STT scalar = tau
            cst[:, base + 6 + mh] = (-tau3[sl] if mh < nmh3 else tau3[sl])
        for mc in range(_KC):
            cst[:, base + 10 + mc] = rw2[mc * 128:(mc + 1) * 128]
    return (w1dr.astype(e4), w2dr.astype(e4), dwp.astype(e4),
            dw2.astype(e4), cst)


def kernel(**inputs):
    inputs = {k: np.asarray(v) for k, v in inputs.items()}
    x = inputs['x'].astype(np.float32)
    w1dr, w2dr, dwp, dw2, cst = _prep(inputs)

    if 'nc' not in _nc_cache:
        _nc_cache['nc'] = _build_nc()
    nc = _nc_cache['nc']

    in_maps = []
    for c in range(_NCORES):
        in_maps.append({
            'x': np.ascontiguousarray(x[c * _BS:(c + 1) * _BS]),
            'w1dr': w1dr, 'w2dr': w2dr, 'dwp': dwp, 'dw2': dw2,
            'cst': cst,
        })

    from concourse.bass_utils import run_bass_kernel_spmd
    import os
    trace = bool(int(os.environ.get('KERNEL_TRACE', '0')))
    res = run_bass_kernel_spmd(nc, in_maps, core_ids=list(range(_NCORES)),
                               trace=trace)
    _nc_cache['last_result'] = res
    out = np.concatenate([r['out'] for r in res.results], axis=0)
    return out.astype(np.float32)



# revision 13
# speedup vs baseline: 1.2624x; 1.2427x over previous
"""Bass/Trainium2 kernel for nn_BitwiseTasNetRepeat.

Strategy (v4c: balanced 3-engine split of the threshold passes)
---------------------------------------------------------------
Every sign(BN(.)) collapses to a per-channel threshold compare. Per block:

    u  = (R >= t1)                          (DVE is_ge, fp8 {0,1};
                                             single-src op -> 2x_2P mode)
    P  = (2*w1s) @ u                        (TensorE fp8 DoubleRow, K=256)
    G  = Sign(-P + t2 + rowsum(w1s))        (ACT, fp8 {-1,0,1});  G = -s2

  depthwise, mh tiles 0..NMH3-1 ("3-tap" path, S3 on ACT):
    q  = -a0*G(-d) - G(0) - a2*G(+d)        (TensorE: DR pair (t-d,t) +
                                             plain tap (t+d), 2 MMs)
    S3 = Sign(q - tau)                      (ACT, fp8 {-1,1})

  depthwise, mh tiles NMH3..3 ("2-tap" path, S3 on DVE):
    q2 = -a0*G(-d) - a2*G(+d)               (TensorE: ONE DR diag matmul,
                                             rhs pair-AP stride 2d)
    S3 = ((q2 - tau) >= G(0))               (DVE scalar_tensor_tensor,
                                             fp8 {0,1}) -- center tap
                                             folded into the compare

    P2 = w2' @ S3                           (TensorE fp8 DR, K=512; w2' is
                                             W2x for 3-tap rows, 2*W2x for
                                             2-tap rows)
    R  = (P2 - rowsum2) + R                 (DVE scalar_tensor_tensor)

with a0 = sign(dw0)*ctr, a2 = sign(dw2)*ctr, ctr = sign(dw1) folded into
W2x = sign(w2)*ctr and tau = ctr*t3.  All values are exact in fp8/f32-PSUM.

Sharding: data-parallel over batch, 2 batches per core, 8 cores.
"""

import numpy as np
import ml_dtypes

_B, _CB, _H, _T = 16, 256, 512, 4096
_BLOCKS = 8
_EPS = 1e-5
_NCORES = 8
_BS = _B // _NCORES      # batches per core
_KC = _CB // 128         # 2  k-tiles of Cb
_MH = _H // 128          # 4  m-tiles of H
_NMH3 = 2                # mh tiles using the 3-tap/ACT-S3 path
_PAD = 128               # halo for dilated depthwise conv (max d = 128)
_NCC = 12                # f32 const columns per block
_QC = 1024               # chunk width for dw/S3/conv2 stages

_nc_cache = {}


def _mk3(ap2d, j_step, cols):
    """3D AP [128, 2 (stride j_step), cols] over a 2D row view."""
    import bass_rust
    v = ap2d.copy()
    l = v.ap
    v.ap = bass_rust.VecI64Pair([list(l[0]), [j_step, 2], [1, cols]])
    return v


def _build_nc(bs=_BS, nblocks=_BLOCKS, T=_T):
    import concourse.mybir as mybir
    from concourse import bacc
    from concourse.tile import TileContext

    f32 = mybir.dt.float32
    fp8 = mybir.dt.float8e4
    ALU = mybir.AluOpType
    DRM = mybir.MatmulPerfMode.DoubleRow
    SIGN = mybir.ActivationFunctionType.Sign
    nq = T // _QC

    nc = bacc.Bacc("TRN2", target_bir_lowering=False, debug=False,
                   enable_asserts=False)

    x_d = nc.dram_tensor("x", [bs, _CB, T], f32, kind="ExternalInput")
    w1_d = nc.dram_tensor("w1dr", [128, nblocks * _MH * 256], fp8,
                          kind="ExternalInput")
    w2_d = nc.dram_tensor("w2dr", [128, nblocks * _KC * 2 * 256], fp8,
                          kind="ExternalInput")
    dwp_d = nc.dram_tensor("dwp", [128, nblocks * _MH * 256], fp8,
                           kind="ExternalInput")
    dw2_d = nc.dram_tensor("dw2", [128, nblocks * _NMH3 * 128], fp8,
                           kind="ExternalInput")
    cst_d = nc.dram_tensor("cst", [128, nblocks * _NCC], f32,
                           kind="ExternalInput")
    out_d = nc.dram_tensor("out", [bs, _CB, T], f32, kind="ExternalOutput")

    with TileContext(nc) as tc:
        with (
            tc.tile_pool(name="wpool", bufs=1) as wpool,
            tc.tile_pool(name="rpool", bufs=4) as rpool,
            tc.tile_pool(name="s1pool", bufs=4) as s1pool,
            tc.tile_pool(name="s2pool", bufs=12) as s2pool,
            tc.tile_pool(name="s3pool", bufs=8) as s3pool,
            tc.tile_pool(name="psmm", bufs=2, space="PSUM") as psmm,
            tc.tile_pool(name="psdw", bufs=2, space="PSUM") as psdw,
        ):
            w1sb = wpool.tile([128, nblocks * _MH * 256], fp8)
            nc.sync.dma_start(out=w1sb[:], in_=w1_d.ap())
            w2sb = wpool.tile([128, nblocks * _KC * 2 * 256], fp8)
            nc.sync.dma_start(out=w2sb[:], in_=w2_d.ap())
            dwpsb = wpool.tile([128, nblocks * _MH * 256], fp8)
            nc.sync.dma_start(out=dwpsb[:], in_=dwp_d.ap())
            dw2sb = wpool.tile([128, nblocks * _NMH3 * 128], fp8)
            nc.sync.dma_start(out=dw2sb[:], in_=dw2_d.ap())
            cst = wpool.tile([128, nblocks * _NCC], f32)
            nc.sync.dma_start(out=cst[:], in_=cst_d.ap())

            def w1t(i, mh):
                o = (i * _MH + mh) * 256
                return _mk3(w1sb[:, o:o + 256], 128, 128)

            def w2t(i, mc, pair):
                o = (i * _KC * 2 + mc * 2 + pair) * 256
                return _mk3(w2sb[:, o:o + 256], 128, 128)

            def dwt(i, mh):
                o = (i * _MH + mh) * 256
                return _mk3(dwpsb[:, o:o + 256], 128, 128)

            def dw2t(i, mh):
                o = (i * _NMH3 + mh) * 128
                return dw2sb[:, o:o + 128]

            def cc(i, j):
                return cst[:, i * _NCC + j:i * _NCC + j + 1]

            # PE warmup: a few junk matmuls so HAM reaches K=8/8 before the
            # real stream starts (weights tile reused as dummy data). Lives
            # in psdw whose first real tile isn't needed until stage BC.
            wu = psdw.tile([128, _QC], f32, tag="dw", name="warmup")
            for _ in range(12):
                nc.tensor.matmul(wu[:, 0:512], w1sb[:, 0:128],
                                 w1sb[:, 0:512], start=True, stop=True)

            Rb = {}
            for b in range(bs):
                Rb[b] = []
                for kc in range(_KC):
                    rt = rpool.tile([128, T], f32, tag="R",
                                    name=f"R_b{b}_{kc}")
                    Rb[b].append(rt)
            # chunk-granular loads, first chunks of every tile up front so
            # the first S1 (needs both kc tiles' chunk 0) starts early
            for g in range(nq):
                for b in range(bs):
                    for kc in range(_KC):
                        nc.sync.dma_start(
                            out=Rb[b][kc][:, g * _QC:(g + 1) * _QC],
                            in_=x_d.ap()[b, kc * 128:(kc + 1) * 128,
                                         g * _QC:(g + 1) * _QC])

            state = {}

            def emitA_alloc(b, i):
                S1 = s1pool.tile([128, _KC * T], fp8, tag="S1",
                                 name=f"S1_b{b}_i{i}")
                G = []
                for mh in range(_MH):
                    gt = s2pool.tile([128, T + 2 * _PAD], fp8, tag="S2",
                                     name=f"G_b{b}_i{i}_{mh}")
                    nc.gpsimd.memset(gt[:, 0:_PAD], 0.0)
                    nc.gpsimd.memset(gt[:, _PAD + T:2 * _PAD + T], 0.0)
                    G.append(gt)
                state[(b, i)] = (S1, G)

            def emitA_g(b, i, g):
                R = Rb[b]
                S1, G = state[(b, i)]
                c0 = g * _QC
                if g % 2 == 0:
                    # u = (R >= t1) in {0,1}; single-src op -> DVE 2x mode.
                    # 2-chunk-wide ops halve the per-instruction overhead
                    # while keeping the R dependency nearly chunk-granular.
                    for kc in range(_KC):
                        nc.vector.tensor_scalar(
                            S1[:, kc * T + c0:kc * T + c0 + 2 * _QC],
                            R[kc][:, c0:c0 + 2 * _QC],
                            cc(i, kc), None, op0=ALU.is_ge)
                for mh in range(_MH):
                    ps = psmm.tile([128, _QC], f32, tag="mm",
                                   name=f"psA_{b}_{i}_{mh}_{g}")
                    for nn in range(2):
                        cn = c0 + nn * 512
                        rhs = _mk3(S1[:, cn:cn + 512], T, 512)
                        nc.tensor.matmul(
                            ps[:, nn * 512:(nn + 1) * 512],
                            w1t(i, mh), rhs, start=True, stop=True,
                            perf_mode=DRM)
                    # G = Sign(-P + t2 + rowsum(w1s)) = -s2
                    nc.scalar.activation(
                        G[mh][:, _PAD + c0:_PAD + c0 + _QC],
                        ps[:], SIGN, bias=cc(i, 2 + mh), scale=-1.0)

            def emitBC_q(b, i, q):
                d = 2 ** i
                R = Rb[b]
                _, G = state[(b, i)]
                c0 = q * _QC
                S3 = [s3pool.tile([128, 2 * _QC], fp8, tag="S3",
                                  name=f"S3_b{b}_i{i}_q{q}_p{p}")
                      for p in range(2)]
                for mh in range(_MH):
                    pd = psdw.tile([128, _QC], f32, tag="dw",
                                   name=f"psD_{b}_{i}_q{q}_{mh}")
                    s3out = S3[mh // 2][:, (mh % 2) * _QC:(mh % 2 + 1) * _QC]
                    if mh < _NMH3:
                        # 3-tap: q = -a0*G(-d) - G(0) - a2*G(+d) in PSUM
                        for nn in range(2):
                            w0 = _PAD + c0 + nn * 512
                            rhs01 = _mk3(G[mh][:, w0 - d:w0 - d + 512], d, 512)
                            nc.tensor.matmul(
                                pd[:, nn * 512:(nn + 1) * 512],
                                dwt(i, mh), rhs01,
                                start=True, stop=False, perf_mode=DRM)
                            nc.tensor.matmul(
                                pd[:, nn * 512:(nn + 1) * 512],
                                dw2t(i, mh),
                                G[mh][:, w0 + d:w0 + d + 512],
                                start=False, stop=True)
                        # S3 = Sign(q - tau) in {-1,1}
                        nc.scalar.activation(
                            s3out, pd[:], SIGN, bias=cc(i, 6 + mh))
                    else:
                        # 2-tap: q2 = -a0*G(-d) - a2*G(+d), one DR matmul
                        for nn in range(2):
                            w0 = _PAD + c0 + nn * 512
                            rhs = _mk3(G[mh][:, w0 - d:w0 - d + 512],
                                       2 * d, 512)
                            nc.tensor.matmul(
                                pd[:, nn * 512:(nn + 1) * 512],
                                dwt(i, mh), rhs,
                                start=True, stop=True, perf_mode=DRM)
                        # S3 = ((q2 - tau) >= G(0)) in {0,1}
                        nc.vector.scalar_tensor_tensor(
                            s3out, pd[:], cc(i, 6 + mh),
                            G[mh][:, _PAD + c0:_PAD + c0 + _QC],
                            op0=ALU.subtract, op1=ALU.is_ge)
                for mc in range(_KC):
                    ps2 = psmm.tile([128, _QC], f32, tag="mm",
                                    name=f"psC_{b}_{i}_q{q}_{mc}")
                    for nn in range(2):
                        for pair in range(2):
                            rhs = _mk3(S3[pair][:, nn * 512:nn * 512 + 512],
                                       _QC, 512)
                            nc.tensor.matmul(
                                ps2[:, nn * 512:(nn + 1) * 512],
                                w2t(i, mc, pair), rhs,
                                start=(pair == 0), stop=(pair == 1),
                                perf_mode=DRM)
                    # R = (P2 - rowsum2) + R
                    nc.vector.scalar_tensor_tensor(
                        R[mc][:, c0:c0 + _QC], ps2[:], cc(i, 10 + mc),
                        R[mc][:, c0:c0 + _QC],
                        op0=ALU.subtract, op1=ALU.add)
                    if i == _BLOCKS - 1:
                        # final block: stream the finished chunk out now so
                        # the output DMA drains during the remaining compute
                        nc.sync.dma_start(
                            out=out_d.ap()[b, mc * 128:(mc + 1) * 128,
                                           c0:c0 + _QC],
                            in_=R[mc][:, c0:c0 + _QC])

            # software-pipelined emission: stage A of step k+1 interleaves
            # chunk-wise with stage B/C of step k so every engine streams
            seq = [(b, i) for i in range(nblocks) for b in range(bs)]
            emitA_alloc(*seq[0])
            for g in range(nq):
                emitA_g(*seq[0], g)
            for k in range(len(seq)):
                if k + 1 < len(seq):
                    emitA_alloc(*seq[k + 1])
                for g in range(nq):
                    # stage A of step k+1 first: its S1/conv1/G chain is
                    # ready (deps are k-1) and unblocks the other engines
                    if k + 1 < len(seq):
                        emitA_g(*seq[k + 1], g)
                    emitBC_q(*seq[k], g)
                state.pop(seq[k])

    nc.finalize()
    return nc


def _prep(inputs, nblocks=_BLOCKS):
    """Host-side weight/threshold preprocessing (tiny tensors only)."""
    e4 = ml_dtypes.float8_e4m3

    def thr(g, bb, m, v):
        return (m - bb * np.sqrt(v + _EPS) / g).astype(np.float32)

    w1dr = np.zeros((128, nblocks * _MH * 256), np.float32)
    w2dr = np.zeros((128, nblocks * _KC * 2 * 256), np.float32)
    dwp = np.zeros((128, nblocks * _MH * 256), np.float32)
    dw2 = np.zeros((128, nblocks * _NMH3 * 128), np.float32)
    cst = np.zeros((128, nblocks * _NCC), np.float32)
    for i in range(nblocks):
        t1 = thr(inputs['bn1_gamma'][i], inputs['bn1_beta'][i],
                 inputs['bn1_mean'][i], inputs['bn1_var'][i])          # [Cb]
        t2 = thr(inputs['bn2_gamma'][i], inputs['bn2_beta'][i],
                 inputs['bn2_mean'][i], inputs['bn2_var'][i])          # [H]
        t3 = thr(inputs['bn3_gamma'][i], inputs['bn3_beta'][i],
                 inputs['bn3_mean'][i], inputs['bn3_var'][i])          # [H]
        W1s = np.sign(inputs['w1'][i]).astype(np.float32)              # [H, Cb]
        W2s = np.sign(inputs['w2'][i]).astype(np.float32)              # [Cb, H]
        dws = np.sign(inputs['dw_w'][i]).astype(np.float32)            # [H, 3]
        ctr = dws[:, 1]
        a0 = dws[:, 0] * ctr
        a2 = dws[:, 2] * ctr
        W2x = W2s * ctr[None, :]                                       # [Cb, H]
        rw1 = W1s.sum(axis=1)                                          # [H]
        # conv2 weight scale: 3-tap rows (S3 in {-1,1}) use W2x; 2-tap rows
        # (S3 in {0,1}) use 2*W2x with a rowsum correction
        hsc = np.where(np.arange(_H) < _NMH3 * 128, 1.0, 2.0)
        W2p = W2x * hsc[None, :]
        rw2 = W2x[:, _NMH3 * 128:].sum(axis=1)                         # [Cb]
        for mh in range(_MH):
            o = (i * _MH + mh) * 256
            for j in range(2):
                # w1dr[p, j*128+f] = 2*W1s[mh*128+f, j*128+p]
                w1dr[:, o + j * 128:o + (j + 1) * 128] = \
                    2.0 * W1s[mh * 128:(mh + 1) * 128,
                              j * 128:(j + 1) * 128].T
        for mc in range(_KC):
            for pair in range(2):
                o = (i * _KC * 2 + mc * 2 + pair) * 256
                for j in range(2):
                    kh = pair * 2 + j
                    w2dr[:, o + j * 128:o + (j + 1) * 128] = \
                        W2p[mc * 128:(mc + 1) * 128,
                            kh * 128:(kh + 1) * 128].T
        for mh in range(_MH):
            sl = slice(mh * 128, (mh + 1) * 128)
            o = (i * _MH + mh) * 256
            if mh < _NMH3:
                # 3-tap: DR pair (t-d, t) weights (-a0, -1); plain (t+d): -a2
                dwp[np.arange(128), o + np.arange(128)] = -a0[sl]
                dwp[np.arange(128), o + 128 + np.arange(128)] = -1.0
                o2 = (i * _NMH3 + mh) * 128
                dw2[np.arange(128), o2 + np.arange(128)] = -a2[sl]
            else:
                # 2-tap: DR pair (t-d, t+d) weights (-a0, -a2)
                dwp[np.arange(128), o + np.arange(128)] = -a0[sl]
                dwp[np.arange(128), o + 128 + np.arange(128)] = -a2[sl]
        base = i * _NCC
        for kc in range(_KC):
            cst[:, base + kc] = t1[kc * 128:(kc + 1) * 128]
        tau3 = ctr * t3
        for mh in range(_MH):
            sl = slice(mh * 128, (mh + 1) * 128)
            cst[:, base + 2 + mh] = t2[sl] + rw1[sl]       # ACT Sign bias
            # 3-tap mh: ACT Sign bias = -tau ; 2-tap mh: STT scalar = tau
            cst[:, base + 6 + mh] = (-tau3[sl] if mh < _NMH3 else tau3[sl])
        for mc in range(_KC):
            cst[:, base + 10 + mc] = rw2[mc * 128:(mc + 1) * 128]
    return (w1dr.astype(e4), w2dr.astype(e4), dwp.astype(e4),
            dw2.astype(e4), cst)


def kernel(**inputs):
    inputs = {k: np.asarray(v) for k, v in inputs.items()}
    x = inputs['x'].astype(np.float32)
    w1dr, w2dr, dwp, dw2, cst = _prep(inputs)

    if 'nc' not in _nc_cache:
        _nc_cache['nc'] = _build_nc()
    nc = _nc_cache['nc']

    in_maps = []
    for c in range(_NCORES):
        in_maps.append({
            'x': np.ascontiguousarray(x[c * _BS:(c + 1) * _BS]),
            'w1dr': w1dr, 'w2dr': w2dr, 'dwp': dwp, 'dw2': dw2,
            'cst': cst,
        })

    from concourse.bass_utils import run_bass_kernel_spmd
    import os
    trace = bool(int(os.environ.get('KERNEL_TRACE', '0')))
    res = run_bass_kernel_spmd(nc, in_maps, core_ids=list(range(_NCORES)),
                               trace=trace)
    _nc_cache['last_result'] = res
    out = np.concatenate([r['out'] for r in res.results], axis=0)
    return out.astype(np.float32)

